# revision 32
# baseline (speedup 1.0000x reference)
"""Trainium2 Bass kernel for the lifted-structure metric loss (nn_Metric_Loss).

Math (reference): for X in {T (text), Z (interleaved text/shape)}:
    D = X @ X.T                      [4096, 4096]
    E = exp(0.5 + D)
    per pair p (rows i=2p, j=2p+1): S[p] = sum(E[{i,j}, :]) - sum(E[{i,j},{i,j}])
    J[p] = relu(log(S[p]) - D[i,j])^2
    loss_X = mean(J) / 2;  total = loss_T + 2 * loss_Z

Sharding (symmetric): E is symmetric, so only the 36 upper-triangle
[512,512] blocks per loss are computed. 72 block-tasks (both losses) are
dealt 9 per core; the host gathers each task's lhs/rhs column blocks into a
per-core input tensor, so the SPMD program is identical across cores and a
slot doesn't know (or care) which loss/block it computes. Per slot the
device emits: ACT-accumulated row sums of E (rows of block i), a PE
ones-vector col-sum of E (rows of block j, used when i != j), and the
2x2-pair-block corrections + positive-pair sims (used when i == j).
The host does the final O(N) assembly + log/relu/square/mean in float64.

Matmuls run in fp8 e4m3 with DoubleRow (2 MACs/cell/cycle); end-to-end
relative error vs the fp32 reference is ~1e-5.
"""

import numpy as np
import ml_dtypes

import concourse.mybir as mybir
import concourse.tile as tile
from concourse import bacc
from concourse.bass import ds
from concourse.bass_utils import run_bass_kernel_spmd

N, D_EMB = 4096, 1024
P_PAIRS = N // 2
NCORES = 8
B = 512                   # block size
NB = N // B               # 8x8 block grid
MT = B // 128             # 4 m-tiles per block
KC = D_EMB // 128         # 8 k-chunks
NSLOTS = 9                # tasks per core
MARGIN = 0.5

# fixed task deal: 36 upper-tri blocks x 2 losses -> 9 per core. Off-diagonal
# tasks occupy slots 0-6 (they carry the col-sum pipeline); diagonal tasks
# (2 per core) sit in slots 7-8 so the kernel's tail dependency chain is the
# short pair-block-correction path, not the colsum chain.
NOFF = 7
_DIAG = [(l, i, i) for l in range(2) for i in range(NB)]
_OFF = [(l, i, j) for l in range(2) for i in range(NB) for j in range(i + 1, NB)]
SLOTS = [
    _OFF[NOFF * c : NOFF * (c + 1)] + _DIAG[2 * c : 2 * (c + 1)]
    for c in range(NCORES)
]

_CACHE = {}


def _build_nc():
    nc = bacc.Bacc(
        "TRN2", target_bir_lowering=False, debug=False, num_devices=NCORES
    )
    f32 = mybir.dt.float32
    bf16 = mybir.dt.bfloat16
    fp8 = mybir.dt.float8e4
    blk = nc.dram_tensor(
        "blk", [NSLOTS, 2, 128, KC, B], fp8, kind="ExternalInput"
    ).ap()
    m2 = nc.dram_tensor("mask2", [128, 128], f32, kind="ExternalInput").ap()
    mij = nc.dram_tensor("maskij", [128, 128], f32, kind="ExternalInput").ap()
    out_rp = nc.dram_tensor(
        "out_rp", [128, NSLOTS * MT], f32, kind="ExternalOutput"
    ).ap()
    out_eb = nc.dram_tensor(
        "out_eb", [128, NSLOTS * MT], f32, kind="ExternalOutput"
    ).ap()
    out_dij = nc.dram_tensor(
        "out_dij", [128, NSLOTS * MT], f32, kind="ExternalOutput"
    ).ap()
    out_cs = nc.dram_tensor(
        "out_cs", [1, NSLOTS * B], f32, kind="ExternalOutput"
    ).ap()

    with tile.TileContext(nc) as tc:
        with (
            tc.tile_pool(name="xb", bufs=3) as xb_pool,
            tc.tile_pool(name="consts", bufs=1) as consts,
            tc.tile_pool(name="psum", bufs=6, space="PSUM") as psum_pool,
            tc.tile_pool(name="cspsum", bufs=2, space="PSUM") as cs_pool,
            tc.tile_pool(name="esc", bufs=3) as esc_pool,
            tc.tile_pool(name="stats", bufs=3) as stats,
        ):
            bias_sb = consts.tile([128, 1], f32, tag="bias")
            nc.vector.memset(bias_sb, MARGIN)
            ones_sb = consts.tile([128, 1], bf16, tag="ones")
            nc.vector.memset(ones_sb, 1.0)
            rp_sb = consts.tile([128, NSLOTS * MT], f32, tag="rp")
            eb_sb = consts.tile([128, NSLOTS * MT], f32, tag="eb")
            nc.vector.memset(eb_sb, 0.0)
            dij_sb = consts.tile([128, NSLOTS * MT], f32, tag="dij")
            nc.vector.memset(dij_sb, 0.0)
            cs_sb = consts.tile([1, NSLOTS * B], f32, tag="cs")
            nc.vector.memset(cs_sb, 0.0)

            # per-slot input tiles; slot 0 is split by k-chunk pairs so the
            # first matmuls start as soon as their chunks land
            xbs = []
            for s in range(NSLOTS):
                xb = xb_pool.tile([128, 2, KC, B], fp8, tag="xb")
                if s == 0:
                    for p in range(KC // 2):
                        nc.sync.dma_start(
                            out=xb[:, :, 2 * p : 2 * p + 2, :],
                            in_=blk[s, :, :, 2 * p : 2 * p + 2, :].rearrange(
                                "two p kc c -> p two kc c"
                            ),
                        )
                elif s in (1, 2):
                    for p in range(2):
                        nc.sync.dma_start(
                            out=xb[:, :, 4 * p : 4 * p + 4, :],
                            in_=blk[s, :, :, 4 * p : 4 * p + 4, :].rearrange(
                                "two p kc c -> p two kc c"
                            ),
                        )
                else:
                    nc.sync.dma_start(
                        out=xb,
                        in_=blk[s].rearrange("two p kc c -> p two kc c"),
                    )
                xbs.append(xb)
            # masks load after the block data (not needed until the diag
            # slots at the end; keeps the head DMA queue clear)
            m2_sb = consts.tile([128, 128], f32, tag="m2")
            nc.sync.dma_start(out=m2_sb, in_=m2)
            mij_sb = consts.tile([128, 128], f32, tag="mij")
            nc.sync.dma_start(out=mij_sb, in_=mij)

            # delayed col-sum emission: the colsum matmul for slot s is
            # emitted after slot s+1's first main matmul group so PE never
            # waits on ACT/GpSimd
            pending = []

            def flush_pending():
                for cst, acc_bf, s in pending:
                    nc.tensor.matmul(cst, ones_sb[:, 0:1], acc_bf)
                    nc.vector.tensor_copy(
                        out=cs_sb[0:1, ds(s * B, B)], in_=cst
                    )
                pending.clear()

            for s in range(NSLOTS):
                xb = xbs[s]
                esc = esc_pool.tile([128, MT, B], mybir.dt.float32, tag="esc")
                for t in range(MT):
                    col = s * MT + t
                    dpsum = psum_pool.tile([128, B], mybir.dt.float32, tag="dps")
                    for k2 in range(KC // 2):
                        nc.tensor.matmul(
                            dpsum,
                            xb[:, 0, 2 * k2 : 2 * k2 + 2, ds(128 * t, 128)],
                            xb[:, 1, 2 * k2 : 2 * k2 + 2, :],
                            start=(k2 == 0),
                            stop=(k2 == KC // 2 - 1),
                            perf_mode=mybir.MatmulPerfMode.DoubleRow,
                        )
                    if t == 0:
                        flush_pending()
                    nc.scalar.activation(
                        esc[:, t, :],
                        dpsum,
                        mybir.ActivationFunctionType.Exp,
                        bias=bias_sb,
                        scale=1.0,
                        accum_out=rp_sb[:, col : col + 1],
                    )
                    if s >= NOFF:
                        # pair-block corrections, diagonal slots only
                        mblk = stats.tile([128, 128], mybir.dt.float32, tag="mblk")
                        nc.vector.tensor_mul(
                            mblk, esc[:, t, ds(128 * t, 128)], m2_sb
                        )
                        nc.vector.reduce_sum(
                            out=eb_sb[:, col : col + 1],
                            in_=mblk,
                            axis=mybir.AxisListType.X,
                        )
                        mblk2 = stats.tile([128, 128], mybir.dt.float32, tag="mblk2")
                        nc.vector.tensor_mul(
                            mblk2, dpsum[:, ds(128 * t, 128)], mij_sb
                        )
                        nc.vector.reduce_sum(
                            out=dij_sb[:, col : col + 1],
                            in_=mblk2,
                            axis=mybir.AxisListType.X,
                        )
                if s < NOFF:
                    # col sums, off-diagonal slots only: accumulate the 4 exp
                    # tiles (add tree split over DVE + idle GpSimd), then one
                    # ones-vector matmul reduces over partitions
                    cst = cs_pool.tile([1, B], mybir.dt.float32, tag="cs")
                    acc01 = stats.tile([128, B], mybir.dt.float32, tag="acc01")
                    nc.vector.tensor_add(acc01, esc[:, 0, :], esc[:, 1, :])
                    acc23 = stats.tile([128, B], mybir.dt.float32, tag="acc23")
                    nc.gpsimd.tensor_add(acc23, esc[:, 2, :], esc[:, 3, :])
                    acc_bf = stats.tile([128, B], bf16, tag="accbf")
                    nc.vector.tensor_add(acc_bf, acc01, acc23)
                    pending.append((cst, acc_bf, s))
            flush_pending()
            nc.sync.dma_start(out=out_rp, in_=rp_sb)
            nc.sync.dma_start(out=out_eb, in_=eb_sb)
            nc.sync.dma_start(out=out_dij, in_=dij_sb)
            nc.sync.dma_start(out=out_cs, in_=cs_sb)
    nc.compile()
    return nc


def _get_nc():
    if "nc" not in _CACHE:
        _CACHE["nc"] = _build_nc()
    return _CACHE["nc"]


def _make_in_maps(text_embeddings, shape_embeddings):
    T = np.asarray(text_embeddings, dtype=np.float32)
    S = np.asarray(shape_embeddings, dtype=np.float32)
    Z = np.empty_like(T)
    Z[0::2] = T[0::2]
    Z[1::2] = S
    # [loss][128 p, KC, N] fp8: row-block p of X^T chunk kc, cols
    Xg = []
    for X in (T, Z):
        XT = np.ascontiguousarray(X.T).astype(ml_dtypes.float8_e4m3)
        Xg.append(XT.reshape(KC, 128, N).transpose(1, 0, 2))  # [128, KC, N]
    r = np.arange(128)
    mask2 = (r[:, None] // 2 == r[None, :] // 2).astype(np.float32)
    maskij = ((r[:, None] % 2 == 0) & (r[None, :] == r[:, None] + 1)).astype(
        np.float32
    )
    in_maps = []
    for c in range(NCORES):
        blk = np.empty((NSLOTS, 2, 128, KC, B), dtype=ml_dtypes.float8_e4m3)
        for s, (l, i, j) in enumerate(SLOTS[c]):
            blk[s, 0] = Xg[l][:, :, B * i : B * (i + 1)]
            blk[s, 1] = Xg[l][:, :, B * j : B * (j + 1)]
        in_maps.append({"blk": blk, "mask2": mask2, "maskij": maskij})
    return in_maps


def _finalize(outs):
    """outs: list of 8 per-core output dicts -> scalar loss."""
    row_s = [np.zeros(N, np.float64) for _ in range(2)]
    dij_all = [np.zeros(N, np.float64) for _ in range(2)]
    for c, o in enumerate(outs):
        rp = np.asarray(o["out_rp"], np.float64)
        eb = np.asarray(o["out_eb"], np.float64)
        dj = np.asarray(o["out_dij"], np.float64)
        cs = np.asarray(o["out_cs"], np.float64).reshape(-1)
        for s, (l, i, j) in enumerate(SLOTS[c]):
            for t in range(MT):
                col = s * MT + t
                g0 = B * i + 128 * t
                row_s[l][g0 : g0 + 128] += rp[:, col]
                if i == j:
                    row_s[l][g0 : g0 + 128] -= eb[:, col]
                    dij_all[l][g0 : g0 + 128] = dj[:, col]
            if i != j:
                row_s[l][B * j : B * (j + 1)] += cs[s * B : (s + 1) * B]
    total = 0.0
    for l in range(2):
        s_pair = row_s[l][0::2] + row_s[l][1::2]
        d_ij = dij_all[l][0::2]
        j_val = np.square(np.maximum(np.log(s_pair) - d_ij, 0.0))
        loss = j_val.sum() / P_PAIRS / 2.0
        total += loss if l == 0 else 2.0 * loss
    return np.asarray(total, dtype=np.float32)


def kernel(text_embeddings, shape_embeddings):
    in_maps = _make_in_maps(text_embeddings, shape_embeddings)
    nc = _get_nc()
    res = run_bass_kernel_spmd(nc, in_maps, core_ids=list(range(NCORES)))
    return _finalize(res.results)


# revision 33
# speedup vs baseline: 1.0023x; 1.0023x over previous
"""Trainium2 Bass kernel for the lifted-structure metric loss (nn_Metric_Loss).

Math (reference): for X in {T (text), Z (interleaved text/shape)}:
    D = X @ X.T                      [4096, 4096]
    E = exp(0.5 + D)
    per pair p (rows i=2p, j=2p+1): S[p] = sum(E[{i,j}, :]) - sum(E[{i,j},{i,j}])
    J[p] = relu(log(S[p]) - D[i,j])^2
    loss_X = mean(J) / 2;  total = loss_T + 2 * loss_Z

Sharding (symmetric): E is symmetric, so only the 36 upper-triangle
[512,512] blocks per loss are computed. 72 block-tasks (both losses) are
dealt 9 per core; the host gathers each task's lhs/rhs column blocks into a
per-core input tensor, so the SPMD program is identical across cores and a
slot doesn't know (or care) which loss/block it computes. Per slot the
device emits: ACT-accumulated row sums of E (rows of block i), a PE
ones-vector col-sum of E (rows of block j, used when i != j), and the
2x2-pair-block corrections + positive-pair sims (used when i == j).
The host does the final O(N) assembly + log/relu/square/mean in float64.

Matmuls run in fp8 e4m3 with DoubleRow (2 MACs/cell/cycle); end-to-end
relative error vs the fp32 reference is ~1e-5.
"""

import numpy as np
import ml_dtypes

import concourse.mybir as mybir
import concourse.tile as tile
from concourse import bacc
from concourse.bass import ds
from concourse.bass_utils import run_bass_kernel_spmd

N, D_EMB = 4096, 1024
P_PAIRS = N // 2
NCORES = 8
B = 512                   # block size
NB = N // B               # 8x8 block grid
MT = B // 128             # 4 m-tiles per block
KC = D_EMB // 128         # 8 k-chunks
NSLOTS = 9                # tasks per core
MARGIN = 0.5

# fixed task deal: 36 upper-tri blocks x 2 losses -> 9 per core. Off-diagonal
# tasks occupy slots 0-6 (they carry the col-sum pipeline); diagonal tasks
# (2 per core) sit in slots 7-8 so the kernel's tail dependency chain is the
# short pair-block-correction path, not the colsum chain.
NOFF = 7
_DIAG = [(l, i, i) for l in range(2) for i in range(NB)]
_OFF = [(l, i, j) for l in range(2) for i in range(NB) for j in range(i + 1, NB)]
SLOTS = [
    _OFF[NOFF * c : NOFF * (c + 1)] + _DIAG[2 * c : 2 * (c + 1)]
    for c in range(NCORES)
]

_CACHE = {}


def _build_nc():
    nc = bacc.Bacc(
        "TRN2", target_bir_lowering=False, debug=False, num_devices=NCORES
    )
    f32 = mybir.dt.float32
    bf16 = mybir.dt.bfloat16
    fp8 = mybir.dt.float8e4
    blk = nc.dram_tensor(
        "blk", [NSLOTS, 2, 128, KC, B], fp8, kind="ExternalInput"
    ).ap()
    m2 = nc.dram_tensor("mask2", [128, 128], f32, kind="ExternalInput").ap()
    mij = nc.dram_tensor("maskij", [128, 128], f32, kind="ExternalInput").ap()
    out_rp = nc.dram_tensor(
        "out_rp", [128, NSLOTS * MT], f32, kind="ExternalOutput"
    ).ap()
    out_eb = nc.dram_tensor(
        "out_eb", [128, NSLOTS * MT], f32, kind="ExternalOutput"
    ).ap()
    out_dij = nc.dram_tensor(
        "out_dij", [128, NSLOTS * MT], f32, kind="ExternalOutput"
    ).ap()
    out_cs = nc.dram_tensor(
        "out_cs", [1, NSLOTS * B], f32, kind="ExternalOutput"
    ).ap()

    with tile.TileContext(nc) as tc:
        with (
            tc.tile_pool(name="xb", bufs=4) as xb_pool,
            tc.tile_pool(name="consts", bufs=1) as consts,
            tc.tile_pool(name="psum", bufs=6, space="PSUM") as psum_pool,
            tc.tile_pool(name="cspsum", bufs=2, space="PSUM") as cs_pool,
            tc.tile_pool(name="esc", bufs=4) as esc_pool,
            tc.tile_pool(name="stats", bufs=3) as stats,
        ):
            bias_sb = consts.tile([128, 1], f32, tag="bias")
            nc.vector.memset(bias_sb, MARGIN)
            ones_sb = consts.tile([128, 1], bf16, tag="ones")
            nc.vector.memset(ones_sb, 1.0)
            rp_sb = consts.tile([128, NSLOTS * MT], f32, tag="rp")
            eb_sb = consts.tile([128, NSLOTS * MT], f32, tag="eb")
            nc.vector.memset(eb_sb, 0.0)
            dij_sb = consts.tile([128, NSLOTS * MT], f32, tag="dij")
            nc.vector.memset(dij_sb, 0.0)
            cs_sb = consts.tile([1, NSLOTS * B], f32, tag="cs")
            nc.vector.memset(cs_sb, 0.0)

            # per-slot input tiles; slot 0 is split by k-chunk pairs so the
            # first matmuls start as soon as their chunks land
            xbs = []
            for s in range(NSLOTS):
                xb = xb_pool.tile([128, 2, KC, B], fp8, tag="xb")
                if s == 0:
                    for p in range(KC // 2):
                        nc.sync.dma_start(
                            out=xb[:, :, 2 * p : 2 * p + 2, :],
                            in_=blk[s, :, :, 2 * p : 2 * p + 2, :].rearrange(
                                "two p kc c -> p two kc c"
                            ),
                        )
                elif s in (1, 2):
                    for p in range(2):
                        nc.sync.dma_start(
                            out=xb[:, :, 4 * p : 4 * p + 4, :],
                            in_=blk[s, :, :, 4 * p : 4 * p + 4, :].rearrange(
                                "two p kc c -> p two kc c"
                            ),
                        )
                else:
                    nc.sync.dma_start(
                        out=xb,
                        in_=blk[s].rearrange("two p kc c -> p two kc c"),
                    )
                xbs.append(xb)
            # masks load after the block data (not needed until the diag
            # slots at the end; keeps the head DMA queue clear)
            m2_sb = consts.tile([128, 128], f32, tag="m2")
            nc.sync.dma_start(out=m2_sb, in_=m2)
            mij_sb = consts.tile([128, 128], f32, tag="mij")
            nc.sync.dma_start(out=mij_sb, in_=mij)

            # delayed col-sum emission: the colsum matmul for slot s is
            # emitted after slot s+1's first main matmul group so PE never
            # waits on ACT/GpSimd
            pending = []

            def flush_pending():
                for cst, acc_bf, s in pending:
                    nc.tensor.matmul(cst, ones_sb[:, 0:1], acc_bf)
                    nc.vector.tensor_copy(
                        out=cs_sb[0:1, ds(s * B, B)], in_=cst
                    )
                pending.clear()

            for s in range(NSLOTS):
                xb = xbs[s]
                esc = esc_pool.tile([128, MT, B], mybir.dt.float32, tag="esc")
                for t in range(MT):
                    col = s * MT + t
                    dpsum = psum_pool.tile([128, B], mybir.dt.float32, tag="dps")
                    for k2 in range(KC // 2):
                        nc.tensor.matmul(
                            dpsum,
                            xb[:, 0, 2 * k2 : 2 * k2 + 2, ds(128 * t, 128)],
                            xb[:, 1, 2 * k2 : 2 * k2 + 2, :],
                            start=(k2 == 0),
                            stop=(k2 == KC // 2 - 1),
                            perf_mode=mybir.MatmulPerfMode.DoubleRow,
                        )
                    if t == 0:
                        flush_pending()
                    nc.scalar.activation(
                        esc[:, t, :],
                        dpsum,
                        mybir.ActivationFunctionType.Exp,
                        bias=bias_sb,
                        scale=1.0,
                        accum_out=rp_sb[:, col : col + 1],
                    )
                    if s >= NOFF:
                        # pair-block corrections, diagonal slots only
                        mblk = stats.tile([128, 128], mybir.dt.float32, tag="mblk")
                        nc.vector.tensor_mul(
                            mblk, esc[:, t, ds(128 * t, 128)], m2_sb
                        )
                        nc.vector.reduce_sum(
                            out=eb_sb[:, col : col + 1],
                            in_=mblk,
                            axis=mybir.AxisListType.X,
                        )
                        mblk2 = stats.tile([128, 128], mybir.dt.float32, tag="mblk2")
                        nc.vector.tensor_mul(
                            mblk2, dpsum[:, ds(128 * t, 128)], mij_sb
                        )
                        nc.vector.reduce_sum(
                            out=dij_sb[:, col : col + 1],
                            in_=mblk2,
                            axis=mybir.AxisListType.X,
                        )
                if s < NOFF:
                    # col sums, off-diagonal slots only: accumulate the 4 exp
                    # tiles (add tree split over DVE + idle GpSimd), then one
                    # ones-vector matmul reduces over partitions
                    cst = cs_pool.tile([1, B], mybir.dt.float32, tag="cs")
                    acc01 = stats.tile([128, B], mybir.dt.float32, tag="acc01")
                    nc.vector.tensor_add(acc01, esc[:, 0, :], esc[:, 1, :])
                    acc23 = stats.tile([128, B], mybir.dt.float32, tag="acc23")
                    nc.gpsimd.tensor_add(acc23, esc[:, 2, :], esc[:, 3, :])
                    acc_bf = stats.tile([128, B], bf16, tag="accbf")
                    nc.vector.tensor_add(acc_bf, acc01, acc23)
                    pending.append((cst, acc_bf, s))
            flush_pending()
            nc.sync.dma_start(out=out_rp, in_=rp_sb)
            nc.sync.dma_start(out=out_eb, in_=eb_sb)
            nc.sync.dma_start(out=out_dij, in_=dij_sb)
            nc.sync.dma_start(out=out_cs, in_=cs_sb)
    nc.compile()
    return nc


def _get_nc():
    if "nc" not in _CACHE:
        _CACHE["nc"] = _build_nc()
    return _CACHE["nc"]


def _make_in_maps(text_embeddings, shape_embeddings):
    T = np.asarray(text_embeddings, dtype=np.float32)
    S = np.asarray(shape_embeddings, dtype=np.float32)
    Z = np.empty_like(T)
    Z[0::2] = T[0::2]
    Z[1::2] = S
    # [loss][128 p, KC, N] fp8: row-block p of X^T chunk kc, cols
    Xg = []
    for X in (T, Z):
        XT = np.ascontiguousarray(X.T).astype(ml_dtypes.float8_e4m3)
        Xg.append(XT.reshape(KC, 128, N).transpose(1, 0, 2))  # [128, KC, N]
    r = np.arange(128)
    mask2 = (r[:, None] // 2 == r[None, :] // 2).astype(np.float32)
    maskij = ((r[:, None] % 2 == 0) & (r[None, :] == r[:, None] + 1)).astype(
        np.float32
    )
    in_maps = []
    for c in range(NCORES):
        blk = np.empty((NSLOTS, 2, 128, KC, B), dtype=ml_dtypes.float8_e4m3)
        for s, (l, i, j) in enumerate(SLOTS[c]):
            blk[s, 0] = Xg[l][:, :, B * i : B * (i + 1)]
            blk[s, 1] = Xg[l][:, :, B * j : B * (j + 1)]
        in_maps.append({"blk": blk, "mask2": mask2, "maskij": maskij})
    return in_maps


def _finalize(outs):
    """outs: list of 8 per-core output dicts -> scalar loss."""
    row_s = [np.zeros(N, np.float64) for _ in range(2)]
    dij_all = [np.zeros(N, np.float64) for _ in range(2)]
    for c, o in enumerate(outs):
        rp = np.asarray(o["out_rp"], np.float64)
        eb = np.asarray(o["out_eb"], np.float64)
        dj = np.asarray(o["out_dij"], np.float64)
        cs = np.asarray(o["out_cs"], np.float64).reshape(-1)
        for s, (l, i, j) in enumerate(SLOTS[c]):
            for t in range(MT):
                col = s * MT + t
                g0 = B * i + 128 * t
                row_s[l][g0 : g0 + 128] += rp[:, col]
                if i == j:
                    row_s[l][g0 : g0 + 128] -= eb[:, col]
                    dij_all[l][g0 : g0 + 128] = dj[:, col]
            if i != j:
                row_s[l][B * j : B * (j + 1)] += cs[s * B : (s + 1) * B]
    total = 0.0
    for l in range(2):
        s_pair = row_s[l][0::2] + row_s[l][1::2]
        d_ij = dij_all[l][0::2]
        j_val = np.square(np.maximum(np.log(s_pair) - d_ij, 0.0))
        loss = j_val.sum() / P_PAIRS / 2.0
        total += loss if l == 0 else 2.0 * loss
    return np.asarray(total, dtype=np.float32)


def kernel(text_embeddings, shape_embeddings):
    in_maps = _make_in_maps(text_embeddings, shape_embeddings)
    nc = _get_nc()
    res = run_bass_kernel_spmd(nc, in_maps, core_ids=list(range(NCORES)))
    return _finalize(res.results)


# revision 40
# speedup vs baseline: 1.0355x; 1.0331x over previous
"""Trainium2 Bass kernel for the lifted-structure metric loss (nn_Metric_Loss).

Math (reference): for X in {T (text), Z (interleaved text/shape)}:
    D = X @ X.T                      [4096, 4096]
    E = exp(0.5 + D)
    per pair p (rows i=2p, j=2p+1): S[p] = sum(E[{i,j}, :]) - sum(E[{i,j},{i,j}])
    J[p] = relu(log(S[p]) - D[i,j])^2
    loss_X = mean(J) / 2;  total = loss_T + 2 * loss_Z

Sharding (symmetric): E is symmetric, so only the 36 upper-triangle
[512,512] blocks per loss are computed. 72 block-tasks (both losses) are
dealt 9 per core; the host gathers each task's lhs/rhs column blocks into a
per-core input tensor, so the SPMD program is identical across cores and a
slot doesn't know (or care) which loss/block it computes. Per slot the
device emits: ACT-accumulated row sums of E (rows of block i), a PE
ones-vector col-sum of E (rows of block j, used when i != j), and the
2x2-pair-block corrections + positive-pair sims (used when i == j).
The host does the final O(N) assembly + log/relu/square/mean in float64.

Matmuls run in fp8 e4m3 with DoubleRow (2 MACs/cell/cycle); end-to-end
relative error vs the fp32 reference is ~1e-5.
"""

import numpy as np
import ml_dtypes

import concourse.mybir as mybir
import concourse.tile as tile
from concourse import bacc
from concourse.bass import ds
from concourse.bass_utils import run_bass_kernel_spmd

N, D_EMB = 4096, 1024
P_PAIRS = N // 2
NCORES = 8
B = 512                   # block size
NB = N // B               # 8x8 block grid
MT = B // 128             # 4 m-tiles per block
KC = D_EMB // 128         # 8 k-chunks
NSLOTS = 9                # tasks per core
MARGIN = 0.5

# fixed task deal: 36 upper-tri blocks x 2 losses -> 9 per core. Off-diagonal
# tasks occupy slots 0-6 (they carry the col-sum pipeline); diagonal tasks
# (2 per core) sit in slots 7-8 so the kernel's tail dependency chain is the
# short pair-block-correction path, not the colsum chain.
NOFF = 7
_DIAG = [(l, i, i) for l in range(2) for i in range(NB)]
_OFF = [(l, i, j) for l in range(2) for i in range(NB) for j in range(i + 1, NB)]
SLOTS = [
    _OFF[NOFF * c : NOFF * (c + 1)] + _DIAG[2 * c : 2 * (c + 1)]
    for c in range(NCORES)
]

_CACHE = {}


def _build_nc():
    nc = bacc.Bacc(
        "TRN2", target_bir_lowering=False, debug=False, num_devices=NCORES
    )
    f32 = mybir.dt.float32
    bf16 = mybir.dt.bfloat16
    fp8 = mybir.dt.float8e4
    blk = nc.dram_tensor(
        "blk", [NSLOTS, 2, 128, KC, B], fp8, kind="ExternalInput"
    ).ap()
    m2 = nc.dram_tensor("mask2", [128, 128], f32, kind="ExternalInput").ap()
    mij = nc.dram_tensor("maskij", [128, 128], f32, kind="ExternalInput").ap()
    # [128, 3*36]: rp | eb | dij column groups
    out_main = nc.dram_tensor(
        "out_main", [128, 3 * NSLOTS * MT], f32, kind="ExternalOutput"
    ).ap()
    # per off-diag slot: sum of the 4 exp tiles; host reduces partitions
    out_acc = nc.dram_tensor(
        "out_acc", [NOFF, 128, B], bf16, kind="ExternalOutput"
    ).ap()

    with tile.TileContext(nc) as tc:
        with (
            tc.tile_pool(name="xb", bufs=4) as xb_pool,
            tc.tile_pool(name="consts", bufs=1) as consts,
            tc.tile_pool(name="psum", bufs=7, space="PSUM") as psum_pool,
            tc.tile_pool(name="esc", bufs=4) as esc_pool,
            tc.tile_pool(name="stats", bufs=3) as stats,
        ):
            bias_sb = consts.tile([128, 1], f32, tag="bias")
            nc.vector.memset(bias_sb, MARGIN)
            main_sb = consts.tile([128, 3 * NSLOTS * MT], f32, tag="main")
            nc.vector.memset(main_sb, 0.0)
            NM = NSLOTS * MT
            rp_sb = main_sb[:, 0:NM]
            eb_sb = main_sb[:, NM : 2 * NM]
            dij_sb = main_sb[:, 2 * NM : 3 * NM]

            # per-slot input tiles; slot 0 is split by k-chunk pairs so the
            # first matmuls start as soon as their chunks land
            xbs = []
            for s in range(NSLOTS):
                xb = xb_pool.tile([128, 2, KC, B], fp8, tag="xb")
                if s == 0:
                    for p in range(KC // 2):
                        nc.sync.dma_start(
                            out=xb[:, :, 2 * p : 2 * p + 2, :],
                            in_=blk[s, :, :, 2 * p : 2 * p + 2, :].rearrange(
                                "two p kc c -> p two kc c"
                            ),
                        )
                elif s in (1, 2):
                    for p in range(2):
                        nc.sync.dma_start(
                            out=xb[:, :, 4 * p : 4 * p + 4, :],
                            in_=blk[s, :, :, 4 * p : 4 * p + 4, :].rearrange(
                                "two p kc c -> p two kc c"
                            ),
                        )
                else:
                    nc.sync.dma_start(
                        out=xb,
                        in_=blk[s].rearrange("two p kc c -> p two kc c"),
                    )
                xbs.append(xb)
            # masks load after the block data (not needed until the diag
            # slots at the end; keeps the head DMA queue clear)
            m2_sb = consts.tile([128, 128], f32, tag="m2")
            nc.sync.dma_start(out=m2_sb, in_=m2)
            mij_sb = consts.tile([128, 128], f32, tag="mij")
            nc.sync.dma_start(out=mij_sb, in_=mij)

            for s in range(NSLOTS):
                xb = xbs[s]
                esc = esc_pool.tile([128, MT, B], mybir.dt.float32, tag="esc")
                for t in range(MT):
                    col = s * MT + t
                    dpsum = psum_pool.tile([128, B], mybir.dt.float32, tag="dps")
                    for k2 in range(KC // 2):
                        nc.tensor.matmul(
                            dpsum,
                            xb[:, 0, 2 * k2 : 2 * k2 + 2, ds(128 * t, 128)],
                            xb[:, 1, 2 * k2 : 2 * k2 + 2, :],
                            start=(k2 == 0),
                            stop=(k2 == KC // 2 - 1),
                            perf_mode=mybir.MatmulPerfMode.DoubleRow,
                        )
                    nc.scalar.activation(
                        esc[:, t, :],
                        dpsum,
                        mybir.ActivationFunctionType.Exp,
                        bias=bias_sb,
                        scale=1.0,
                        accum_out=rp_sb[:, col : col + 1],
                    )
                    if s >= NOFF:
                        # pair-block corrections, diagonal slots only
                        mblk = stats.tile([128, 128], mybir.dt.float32, tag="mblk")
                        nc.vector.tensor_mul(
                            mblk, esc[:, t, ds(128 * t, 128)], m2_sb
                        )
                        nc.vector.reduce_sum(
                            out=eb_sb[:, col : col + 1],
                            in_=mblk,
                            axis=mybir.AxisListType.X,
                        )
                        mblk2 = stats.tile([128, 128], mybir.dt.float32, tag="mblk2")
                        nc.vector.tensor_mul(
                            mblk2, dpsum[:, ds(128 * t, 128)], mij_sb
                        )
                        nc.vector.reduce_sum(
                            out=dij_sb[:, col : col + 1],
                            in_=mblk2,
                            axis=mybir.AxisListType.X,
                        )
                if s < NOFF:
                    # col-sum prep, off-diagonal slots only: accumulate the 4
                    # exp tiles (add tree split over DVE + idle GpSimd) and
                    # stream to DRAM; the host reduces over partitions
                    acc01 = stats.tile([128, B], mybir.dt.float32, tag="acc01")
                    nc.vector.tensor_add(acc01, esc[:, 0, :], esc[:, 1, :])
                    acc23 = stats.tile([128, B], mybir.dt.float32, tag="acc23")
                    nc.gpsimd.tensor_add(acc23, esc[:, 2, :], esc[:, 3, :])
                    acc_bf = stats.tile([128, B], bf16, tag="accbf")
                    nc.vector.tensor_add(acc_bf, acc01, acc23)
                    nc.sync.dma_start(out=out_acc[s], in_=acc_bf)
            nc.sync.dma_start(out=out_main, in_=main_sb)
    nc.compile()
    return nc


def _get_nc():
    if "nc" not in _CACHE:
        _CACHE["nc"] = _build_nc()
    return _CACHE["nc"]


def _make_in_maps(text_embeddings, shape_embeddings):
    T = np.asarray(text_embeddings, dtype=np.float32)
    S = np.asarray(shape_embeddings, dtype=np.float32)
    Z = np.empty_like(T)
    Z[0::2] = T[0::2]
    Z[1::2] = S
    # [loss][128 p, KC, N] fp8: row-block p of X^T chunk kc, cols
    Xg = []
    for X in (T, Z):
        XT = np.ascontiguousarray(X.T).astype(ml_dtypes.float8_e4m3)
        Xg.append(XT.reshape(KC, 128, N).transpose(1, 0, 2))  # [128, KC, N]
    r = np.arange(128)
    mask2 = (r[:, None] // 2 == r[None, :] // 2).astype(np.float32)
    maskij = ((r[:, None] % 2 == 0) & (r[None, :] == r[:, None] + 1)).astype(
        np.float32
    )
    in_maps = []
    for c in range(NCORES):
        blk = np.empty((NSLOTS, 2, 128, KC, B), dtype=ml_dtypes.float8_e4m3)
        for s, (l, i, j) in enumerate(SLOTS[c]):
            blk[s, 0] = Xg[l][:, :, B * i : B * (i + 1)]
            blk[s, 1] = Xg[l][:, :, B * j : B * (j + 1)]
        in_maps.append({"blk": blk, "mask2": mask2, "maskij": maskij})
    return in_maps


def _finalize(outs):
    """outs: list of 8 per-core output dicts -> scalar loss."""
    row_s = [np.zeros(N, np.float64) for _ in range(2)]
    dij_all = [np.zeros(N, np.float64) for _ in range(2)]
    nm = NSLOTS * MT
    for c, o in enumerate(outs):
        main = np.asarray(o["out_main"], np.float64)
        rp = main[:, 0:nm]
        eb = main[:, nm : 2 * nm]
        dj = main[:, 2 * nm : 3 * nm]
        # col sums: reduce the shipped per-slot exp-sum tiles over partitions
        cs = np.asarray(o["out_acc"], np.float32).astype(np.float64).sum(axis=1)
        for s, (l, i, j) in enumerate(SLOTS[c]):
            for t in range(MT):
                col = s * MT + t
                g0 = B * i + 128 * t
                row_s[l][g0 : g0 + 128] += rp[:, col]
                if i == j:
                    row_s[l][g0 : g0 + 128] -= eb[:, col]
                    dij_all[l][g0 : g0 + 128] = dj[:, col]
            if i != j:
                row_s[l][B * j : B * (j + 1)] += cs[s]
    total = 0.0
    for l in range(2):
        s_pair = row_s[l][0::2] + row_s[l][1::2]
        d_ij = dij_all[l][0::2]
        j_val = np.square(np.maximum(np.log(s_pair) - d_ij, 0.0))
        loss = j_val.sum() / P_PAIRS / 2.0
        total += loss if l == 0 else 2.0 * loss
    return np.asarray(total, dtype=np.float32)


def kernel(text_embeddings, shape_embeddings):
    in_maps = _make_in_maps(text_embeddings, shape_embeddings)
    nc = _get_nc()
    res = run_bass_kernel_spmd(nc, in_maps, core_ids=list(range(NCORES)))
    return _finalize(res.results)


# revision 44
# speedup vs baseline: 1.0475x; 1.0116x over previous
"""Trainium2 Bass kernel for the lifted-structure metric loss (nn_Metric_Loss).

Math (reference): for X in {T (text), Z (interleaved text/shape)}:
    D = X @ X.T                      [4096, 4096]
    E = exp(0.5 + D)
    per pair p (rows i=2p, j=2p+1): S[p] = sum(E[{i,j}, :]) - sum(E[{i,j},{i,j}])
    J[p] = relu(log(S[p]) - D[i,j])^2
    loss_X = mean(J) / 2;  total = loss_T + 2 * loss_Z

Sharding (symmetric): E is symmetric, so only the 36 upper-triangle
[512,512] blocks per loss are computed. 72 block-tasks (both losses) are
dealt 9 per core; the host gathers each task's lhs/rhs column blocks into a
per-core input tensor, so the SPMD program is identical across cores and a
slot doesn't know (or care) which loss/block it computes. Per slot the
device emits: ACT-accumulated row sums of E (rows of block i), a PE
ones-vector col-sum of E (rows of block j, used when i != j), and the
2x2-pair-block corrections + positive-pair sims (used when i == j).
The host does the final O(N) assembly + log/relu/square/mean in float64.

Matmuls run in fp8 e4m3 with DoubleRow (2 MACs/cell/cycle); end-to-end
relative error vs the fp32 reference is ~1e-5.
"""

import numpy as np
import ml_dtypes

import concourse.mybir as mybir
import concourse.tile as tile
from concourse import bacc
from concourse.bass import ds
from concourse.bass_utils import run_bass_kernel_spmd

N, D_EMB = 4096, 1024
P_PAIRS = N // 2
NCORES = 8
B = 512                   # block size
NB = N // B               # 8x8 block grid
MT = B // 128             # 4 m-tiles per block
KC = D_EMB // 128         # 8 k-chunks
NSLOTS = 9                # tasks per core
MARGIN = 0.5

# fixed task deal: 36 upper-tri blocks x 2 losses -> 9 per core. Diagonal
# tasks (2 per core) sit at slots 6-7 so their serial DVE mask chains overlap
# slot 8's matmuls; the final slot is off-diagonal, whose tail chain (exp ->
# two DVE adds -> DMA) is the shortest. Off-diag slots: 0-5 and 8.
NOFF = 7
DIAG_SLOTS = (6, 7)
ACC_IDX = {0: 0, 1: 1, 2: 2, 3: 3, 4: 4, 5: 5, 8: 6}  # off-diag slot -> acc row
_DIAG = [(l, i, i) for l in range(2) for i in range(NB)]
_OFF = [(l, i, j) for l in range(2) for i in range(NB) for j in range(i + 1, NB)]
SLOTS = [
    _OFF[7 * c : 7 * c + 6] + _DIAG[2 * c : 2 * (c + 1)] + [_OFF[7 * c + 6]]
    for c in range(NCORES)
]

_CACHE = {}


def _build_nc():
    nc = bacc.Bacc(
        "TRN2", target_bir_lowering=False, debug=False, num_devices=NCORES
    )
    f32 = mybir.dt.float32
    bf16 = mybir.dt.bfloat16
    fp8 = mybir.dt.float8e4
    blk = nc.dram_tensor(
        "blk", [NSLOTS, 2, 128, KC, B], fp8, kind="ExternalInput"
    ).ap()
    m2 = nc.dram_tensor("mask2", [128, 128], f32, kind="ExternalInput").ap()
    mij = nc.dram_tensor("maskij", [128, 128], f32, kind="ExternalInput").ap()
    # [128, 3*36]: rp | eb | dij column groups
    out_main = nc.dram_tensor(
        "out_main", [128, 3 * NSLOTS * MT], f32, kind="ExternalOutput"
    ).ap()
    # per off-diag slot: sum of the 4 exp tiles; host reduces partitions
    out_acc = nc.dram_tensor(
        "out_acc", [NOFF, 128, B], bf16, kind="ExternalOutput"
    ).ap()

    with tile.TileContext(nc) as tc:
        with (
            tc.tile_pool(name="xb", bufs=4) as xb_pool,
            tc.tile_pool(name="consts", bufs=1) as consts,
            tc.tile_pool(name="psum", bufs=7, space="PSUM") as psum_pool,
            tc.tile_pool(name="esc", bufs=4) as esc_pool,
            tc.tile_pool(name="stats", bufs=3) as stats,
        ):
            bias_sb = consts.tile([128, 1], f32, tag="bias")
            nc.vector.memset(bias_sb, MARGIN)
            main_sb = consts.tile([128, 3 * NSLOTS * MT], f32, tag="main")
            nc.vector.memset(main_sb, 0.0)
            NM = NSLOTS * MT
            rp_sb = main_sb[:, 0:NM]
            eb_sb = main_sb[:, NM : 2 * NM]
            dij_sb = main_sb[:, 2 * NM : 3 * NM]

            # per-slot input tiles; slot 0 is split by k-chunk pairs so the
            # first matmuls start as soon as their chunks land
            xbs = []
            for s in range(NSLOTS):
                xb = xb_pool.tile([128, 2, KC, B], fp8, tag="xb")
                if s == 0:
                    for p in range(KC // 2):
                        nc.sync.dma_start(
                            out=xb[:, :, 2 * p : 2 * p + 2, :],
                            in_=blk[s, :, :, 2 * p : 2 * p + 2, :].rearrange(
                                "two p kc c -> p two kc c"
                            ),
                        )
                elif s in (1, 2):
                    for p in range(2):
                        nc.sync.dma_start(
                            out=xb[:, :, 4 * p : 4 * p + 4, :],
                            in_=blk[s, :, :, 4 * p : 4 * p + 4, :].rearrange(
                                "two p kc c -> p two kc c"
                            ),
                        )
                else:
                    nc.sync.dma_start(
                        out=xb,
                        in_=blk[s].rearrange("two p kc c -> p two kc c"),
                    )
                xbs.append(xb)
            # masks load after the block data (not needed until the diag
            # slots at the end; keeps the head DMA queue clear)
            m2_sb = consts.tile([128, 128], f32, tag="m2")
            nc.sync.dma_start(out=m2_sb, in_=m2)
            mij_sb = consts.tile([128, 128], f32, tag="mij")
            nc.sync.dma_start(out=mij_sb, in_=mij)

            for s in range(NSLOTS):
                xb = xbs[s]
                esc = esc_pool.tile([128, MT, B], mybir.dt.float32, tag="esc")
                for t in range(MT):
                    col = s * MT + t
                    dpsum = psum_pool.tile([128, B], mybir.dt.float32, tag="dps")
                    for k2 in range(KC // 2):
                        nc.tensor.matmul(
                            dpsum,
                            xb[:, 0, 2 * k2 : 2 * k2 + 2, ds(128 * t, 128)],
                            xb[:, 1, 2 * k2 : 2 * k2 + 2, :],
                            start=(k2 == 0),
                            stop=(k2 == KC // 2 - 1),
                            perf_mode=mybir.MatmulPerfMode.DoubleRow,
                        )
                    nc.scalar.activation(
                        esc[:, t, :],
                        dpsum,
                        mybir.ActivationFunctionType.Exp,
                        bias=bias_sb,
                        scale=1.0,
                        accum_out=rp_sb[:, col : col + 1],
                    )
                    if s in DIAG_SLOTS:
                        # pair-block corrections, diagonal slots only
                        mblk = stats.tile([128, 128], mybir.dt.float32, tag="mblk")
                        nc.vector.tensor_mul(
                            mblk, esc[:, t, ds(128 * t, 128)], m2_sb
                        )
                        nc.vector.reduce_sum(
                            out=eb_sb[:, col : col + 1],
                            in_=mblk,
                            axis=mybir.AxisListType.X,
                        )
                        mblk2 = stats.tile([128, 128], mybir.dt.float32, tag="mblk2")
                        nc.vector.tensor_mul(
                            mblk2, dpsum[:, ds(128 * t, 128)], mij_sb
                        )
                        nc.vector.reduce_sum(
                            out=dij_sb[:, col : col + 1],
                            in_=mblk2,
                            axis=mybir.AxisListType.X,
                        )
                if s not in DIAG_SLOTS:
                    # col-sum prep, off-diagonal slots only: accumulate the 4
                    # exp tiles (add tree split over DVE + idle GpSimd) and
                    # stream to DRAM; the host reduces over partitions. The
                    # last slot's tree stays on DVE (GpSimd adds are 3x
                    # slower and would sit on the kernel's tail).
                    acc23_eng = nc.vector if s == NSLOTS - 1 else nc.gpsimd
                    acc01 = stats.tile([128, B], mybir.dt.float32, tag="acc01")
                    nc.vector.tensor_add(acc01, esc[:, 0, :], esc[:, 1, :])
                    acc23 = stats.tile([128, B], mybir.dt.float32, tag="acc23")
                    acc23_eng.tensor_add(acc23, esc[:, 2, :], esc[:, 3, :])
                    acc_bf = stats.tile([128, B], bf16, tag="accbf")
                    nc.vector.tensor_add(acc_bf, acc01, acc23)
                    nc.sync.dma_start(out=out_acc[ACC_IDX[s]], in_=acc_bf)
            nc.sync.dma_start(out=out_main, in_=main_sb)
    nc.compile()
    return nc


def _get_nc():
    if "nc" not in _CACHE:
        _CACHE["nc"] = _build_nc()
    return _CACHE["nc"]


def _make_in_maps(text_embeddings, shape_embeddings):
    T = np.asarray(text_embeddings, dtype=np.float32)
    S = np.asarray(shape_embeddings, dtype=np.float32)
    Z = np.empty_like(T)
    Z[0::2] = T[0::2]
    Z[1::2] = S
    # [loss][128 p, KC, N] fp8: row-block p of X^T chunk kc, cols
    Xg = []
    for X in (T, Z):
        XT = np.ascontiguousarray(X.T).astype(ml_dtypes.float8_e4m3)
        Xg.append(XT.reshape(KC, 128, N).transpose(1, 0, 2))  # [128, KC, N]
    r = np.arange(128)
    mask2 = (r[:, None] // 2 == r[None, :] // 2).astype(np.float32)
    maskij = ((r[:, None] % 2 == 0) & (r[None, :] == r[:, None] + 1)).astype(
        np.float32
    )
    in_maps = []
    for c in range(NCORES):
        blk = np.empty((NSLOTS, 2, 128, KC, B), dtype=ml_dtypes.float8_e4m3)
        for s, (l, i, j) in enumerate(SLOTS[c]):
            blk[s, 0] = Xg[l][:, :, B * i : B * (i + 1)]
            blk[s, 1] = Xg[l][:, :, B * j : B * (j + 1)]
        in_maps.append({"blk": blk, "mask2": mask2, "maskij": maskij})
    return in_maps


def _finalize(outs):
    """outs: list of 8 per-core output dicts -> scalar loss."""
    row_s = [np.zeros(N, np.float64) for _ in range(2)]
    dij_all = [np.zeros(N, np.float64) for _ in range(2)]
    nm = NSLOTS * MT
    for c, o in enumerate(outs):
        main = np.asarray(o["out_main"], np.float64)
        rp = main[:, 0:nm]
        eb = main[:, nm : 2 * nm]
        dj = main[:, 2 * nm : 3 * nm]
        # col sums: reduce the shipped per-slot exp-sum tiles over partitions
        cs = np.asarray(o["out_acc"], np.float32).astype(np.float64).sum(axis=1)
        for s, (l, i, j) in enumerate(SLOTS[c]):
            for t in range(MT):
                col = s * MT + t
                g0 = B * i + 128 * t
                row_s[l][g0 : g0 + 128] += rp[:, col]
                if i == j:
                    row_s[l][g0 : g0 + 128] -= eb[:, col]
                    dij_all[l][g0 : g0 + 128] = dj[:, col]
            if i != j:
                row_s[l][B * j : B * (j + 1)] += cs[ACC_IDX[s]]
    total = 0.0
    for l in range(2):
        s_pair = row_s[l][0::2] + row_s[l][1::2]
        d_ij = dij_all[l][0::2]
        j_val = np.square(np.maximum(np.log(s_pair) - d_ij, 0.0))
        loss = j_val.sum() / P_PAIRS / 2.0
        total += loss if l == 0 else 2.0 * loss
    return np.asarray(total, dtype=np.float32)


def kernel(text_embeddings, shape_embeddings):
    in_maps = _make_in_maps(text_embeddings, shape_embeddings)
    nc = _get_nc()
    res = run_bass_kernel_spmd(nc, in_maps, core_ids=list(range(NCORES)))
    return _finalize(res.results)
